# revision 20
# baseline (speedup 1.0000x reference)
"""MoE (top-2 of 8 experts + shared expert) Trainium2 kernel, expert-parallel
across 8 NeuronCores.

Strategy:
  - Host: compute the (tiny) gate in float64 numpy, select top-2 experts per
    token, and dispatch tokens by routing index (the all-to-all of
    expert-parallel MoE, done during the host-side shard step).
  - The shared expert is treated as a 9th expert whose token list is all T
    tokens with combine weight 1.0.  Every core runs the same fixed slot
    capacities (default 896+512+256 = 1664 tokens/core vs the 1536 floor);
    expert token lists are cut into pieces and packed into the 3x8 slots by
    an exact cover search, so hot experts span multiple cores and every core
    does identical compute.
  - Device (per core): feature-major MLP per slot, all matmul operands in
    fp16 (halves DMA vs fp32, enables fast-weight-load on the PE, keeps
    ~5e-4 rel err).  x^T tiles stay resident in SBUF; weights stream i-tile
    by i-tile with a 4-deep prefetch pool; swiglu fused into 6 DVE + 3 ACT
    ops per tile; h stays resident in fp16; the second GEMM accumulates over
    16 i-tiles in fp32 PSUM.
  - Host: combine = scatter-add of per-piece outputs weighted by the gate
    probabilities (1.0 for shared pieces). The swiglu even/odd interleave
    split, transposes, and the 1/1.702 silu rescale are pre-folded into the
    host-side weight layouts.
"""
import sys

sys.path.insert(0, "/opt/trn_rl_repo")

import numpy as np

import concourse.bacc as bacc_mod
import concourse.tile as tile
from concourse import mybir
from concourse.bass_utils import run_bass_kernel_spmd

F32 = mybir.dt.float32
F16 = mybir.dt.float16
Alu = mybir.AluOpType
Act = mybir.ActivationFunctionType

ALPHA = 1.702
LIMIT = 7.0
TOPK = 2
D, I, E = 1024, 2048, 8
B, S = 2, 2048
T = B * S
DK = D // 128          # 8 output d-tiles
IT = I // 128          # 16 i-tiles
N_CORES = 8

# slot capacity tuples to try, tightest first (sum = tokens streamed per core)
CAPS_CANDIDATES = [
    (320, 352, 384, 512),  # 1568/core; fits the seed-0 routing counts
    (704, 512, 384),
    (896, 512, 256),
    (896, 512, 384),
    (1024, 512, 256),
    (1024, 512, 384),
    (1024, 640, 384),
    (1024, 768, 512),
    (1280, 768, 512),
    (1536, 1024, 512),
    (2048, 1024, 512),
    (3072, 2048, 1024),
    (4096, 4096, 512),   # degenerate worst cases
]

_kernel_cache = {}


def _token_groups(n):
    """Split n into matmul token groups of <=512 (one PSUM bank of fp32
    accumulator), each >=256 so weight loads stay hidden under the stream."""
    groups = []
    while n > 512:
        g = 512 if n - 512 >= 256 else n - 256
        groups.append(g)
        n -= g
    groups.append(n)
    return groups


def _build(caps):
    """Build the SPMD Bass kernel; caps = token capacity per slot."""
    nc = bacc_mod.Bacc("TRN2")

    def dram(name, shape, dtype=F16, out=False):
        return nc.declare_dram_parameter(name, list(shape), dtype, isOutput=out)

    slots = []
    for s, cap in enumerate(caps):
        pref = f"s{s}"
        w = {
            "xt": dram(pref + "xt", [128, DK * cap]),
            "w1e": dram(pref + "w1e", [IT, 128, DK, 128]),
            "w1o": dram(pref + "w1o", [IT, 128, DK, 128]),
            "w3e": dram(pref + "w3e", [IT, 128, DK, 128]),
            "w3o": dram(pref + "w3o", [IT, 128, DK, 128]),
            "w2": dram(pref + "w2", [DK, 128, IT, 128]),
            "b1e": dram(pref + "b1e", [128, IT], F32),
            "b1o": dram(pref + "b1o", [128, IT], F32),
            "b3e": dram(pref + "b3e", [128, IT], F32),
            "b3o": dram(pref + "b3o", [128, IT], F32),
            "b2": dram(pref + "b2", [128, DK], F32),
            "y": dram(pref + "y", [DK, 128, cap], F32, out=True),
        }
        slots.append((pref, cap, w))

    with tile.TileContext(nc) as tc:
        with (
            tc.tile_pool(name="persist", bufs=1) as persist,
            tc.tile_pool(name="wpool", bufs=4) as wpool,
            tc.tile_pool(name="w2pool", bufs=3) as w2pool,
            tc.tile_pool(name="work", bufs=2) as work,
            tc.tile_pool(name="outp", bufs=3) as outp,
            tc.tile_pool(name="ps", bufs=1, space="PSUM") as ps,
            tc.tile_pool(name="psy", bufs=4, space="PSUM") as psy,
        ):
            def phase(pref, t_tot, w):
                groups = _token_groups(t_tot)
                offs = np.cumsum([0] + groups)[:-1]

                # x^T for the whole slot: host-packed [128, DK*t_tot], one
                # contiguous DMA descriptor (fast issue at kernel start)
                xts = persist.tile([128, DK * t_tot], F16, tag=f"xt_{pref}",
                                   name=f"xt_{pref}")
                nc.sync.dma_start(out=xts, in_=w["xt"][:, :])

                def load_w13(it):
                    ws = {}
                    for wn in ("w1e", "w3e", "w1o", "w3o"):
                        wt = wpool.tile([128, DK * 128], F16, tag=wn,
                                        name=f"{wn}_{pref}_{it}")
                        nc.sync.dma_start(
                            out=wt.rearrange("p (k i) -> p k i", k=DK),
                            in_=w[wn][it])
                        ws[wn] = wt
                    return ws

                # issue it=0 weight loads before the (less urgent) biases
                ws0 = load_w13(0)

                bias = {}
                for bn in ("b1e", "b1o", "b3e", "b3o"):
                    bt = persist.tile([128, IT], F32, tag=f"{bn}_{pref}",
                                      name=f"{bn}_{pref}")
                    nc.sync.dma_start(out=bt, in_=w[bn][:, :])
                    bias[bn] = bt
                b2t = persist.tile([128, DK], F32, tag=f"b2_{pref}",
                                   name=f"b2_{pref}")
                nc.sync.dma_start(out=b2t, in_=w["b2"][:, :])

                # per-i-tile h tiles so second-GEMM deps are fine-grained
                hbufs = [persist.tile([128, t_tot], F16, tag=f"h_{pref}_{it}",
                                      name=f"h_{pref}_{it}")
                         for it in range(IT)]

                # ---- first GEMM + swiglu: h[i, t] for all i-tiles ----
                for it in range(IT):
                    ws = ws0 if it == 0 else load_w13(it)
                    for g, (goff, gsz) in enumerate(zip(offs, groups)):
                        def mm_acc(tag, wt):
                            acc = ps.tile([128, 512], F32, tag=tag,
                                          name=f"{tag}_{pref}_{it}_{g}")
                            for dk in range(DK):
                                nc.tensor.matmul(
                                    acc[:, :gsz],
                                    wt[:, dk * 128:(dk + 1) * 128],
                                    xts[:, dk * t_tot + goff:
                                        dk * t_tot + goff + gsz],
                                    start=(dk == 0), stop=(dk == DK - 1))
                            return acc

                        A = mm_acc("A", ws["w1e"])
                        Bm = mm_acc("B", ws["w3e"])
                        C = mm_acc("C", ws["w1o"])
                        Dm = mm_acc("D", ws["w3o"])

                        Bp = work.tile([128, 512], F32, tag="Bp")
                        nc.scalar.activation(Bp[:, :gsz], Bm[:, :gsz],
                                             Act.Identity,
                                             bias=bias["b3e"][:, it:it + 1])
                        G = work.tile([128, 512], F32, tag="G")
                        nc.vector.scalar_tensor_tensor(
                            G[:, :gsz], A[:, :gsz], bias["b1e"][:, it:it + 1],
                            Bp[:, :gsz], Alu.add, Alu.mult)
                        nc.vector.tensor_scalar_min(G[:, :gsz], G[:, :gsz], LIMIT)
                        Sg = work.tile([128, 512], F32, tag="Sg")
                        nc.scalar.activation(Sg[:, :gsz], G[:, :gsz],
                                             Act.Sigmoid, scale=ALPHA)
                        # Sv = alpha*G*sigmoid(alpha*G)  (silu(alpha*G))
                        Sv = work.tile([128, 512], F32, tag="Sv")
                        nc.vector.scalar_tensor_tensor(
                            Sv[:, :gsz], G[:, :gsz], ALPHA, Sg[:, :gsz],
                            Alu.mult, Alu.mult)
                        Dp = work.tile([128, 512], F32, tag="Dp")
                        nc.scalar.activation(Dp[:, :gsz], Dm[:, :gsz],
                                             Act.Identity,
                                             bias=bias["b3o"][:, it:it + 1])
                        L = work.tile([128, 512], F32, tag="L")
                        nc.vector.scalar_tensor_tensor(
                            L[:, :gsz], C[:, :gsz], bias["b1o"][:, it:it + 1],
                            Dp[:, :gsz], Alu.add, Alu.mult)
                        nc.vector.tensor_scalar(L[:, :gsz], L[:, :gsz],
                                                LIMIT, -LIMIT, Alu.min, Alu.max)
                        # h = (L + 1) * silu(alpha*G); the 1/alpha rescale is
                        # folded into w2 on the host
                        nc.vector.scalar_tensor_tensor(
                            hbufs[it][:, goff:goff + gsz],
                            L[:, :gsz], 1.0, Sv[:, :gsz], Alu.add, Alu.mult)

                # ---- second GEMM: y[dk] = sum_it w2[dk,it].T @ h[it] ----
                for dk in range(DK):
                    w2t = w2pool.tile([128, IT * 128], F16, tag="w2",
                                      name=f"w2_{pref}_{dk}")
                    nc.sync.dma_start(
                        out=w2t.rearrange("p (n j) -> p n j", n=IT),
                        in_=w["w2"][dk])
                    for g, (goff, gsz) in enumerate(zip(offs, groups)):
                        Y = psy.tile([128, 512], F32, tag="Y",
                                     name=f"Y_{pref}_{dk}_{g}")
                        for it in range(IT):
                            nc.tensor.matmul(
                                Y[:, :gsz],
                                w2t[:, it * 128:(it + 1) * 128],
                                hbufs[it][:, goff:goff + gsz],
                                start=(it == 0), stop=(it == IT - 1))
                        yo = outp.tile([128, 512], F32, tag="yo")
                        nc.scalar.activation(yo[:, :gsz], Y[:, :gsz],
                                             Act.Identity,
                                             bias=b2t[:, dk:dk + 1])
                        nc.sync.dma_start(
                            out=w["y"][dk, :, goff:goff + gsz],
                            in_=yo[:, :gsz])

            for pref, cap, w in slots:
                phase(pref, cap, w)

    nc.finalize()
    return nc


def _tile_w13(wmat):
    """[D, I] -> [IT, 128, DK, 128] (it, d%128, dk, i%128), contiguous fp16."""
    return np.ascontiguousarray(
        wmat.astype(np.float16).reshape(DK, 128, IT, 128).transpose(2, 1, 0, 3))


def _tile_w2(wmat):
    """[I, D] -> [DK, 128, IT, 128] (dk, i%128, it, d%128), contiguous fp16
    so each [dk] slice DMAs as one contiguous run per partition."""
    return np.ascontiguousarray(
        wmat.astype(np.float16).reshape(IT, 128, DK, 128).transpose(2, 1, 0, 3))


def _expert_pack(w1, b1, w3, b3, w2, b2):
    """Split swiglu interleave on the host and tile for DMA."""
    return {
        "w1e": _tile_w13(w1[:, 0::2]),
        "w1o": _tile_w13(w1[:, 1::2]),
        "w3e": _tile_w13(w3[:, 0::2]),
        "w3o": _tile_w13(w3[:, 1::2]),
        "w2": _tile_w2(w2 * np.float32(1.0 / ALPHA)),
        "b1e": np.ascontiguousarray(b1[0::2].reshape(IT, 128).T,
                                    dtype=np.float32),
        "b1o": np.ascontiguousarray(b1[1::2].reshape(IT, 128).T,
                                    dtype=np.float32),
        "b3e": np.ascontiguousarray(b3[0::2].reshape(IT, 128).T,
                                    dtype=np.float32),
        "b3o": np.ascontiguousarray(b3[1::2].reshape(IT, 128).T,
                                    dtype=np.float32),
        "b2": np.ascontiguousarray(b2.reshape(DK, 128).T, dtype=np.float32),
    }


def _xt_pack(xsub16, cap):
    """[n, D] fp16 tokens -> zero-padded [128, DK*cap] transposed layout
    (partition = d%128, free = dk-major then token)."""
    n = xsub16.shape[0]
    xt = np.zeros((D, cap), dtype=np.float16)
    xt[:, :n] = xsub16.T
    return np.ascontiguousarray(
        xt.reshape(DK, 128, cap).transpose(1, 0, 2).reshape(128, DK * cap))


def _solve_cover(counts, caps):
    """Assign pieces (8 per slot class) to experts so each expert's total
    capacity >= its count. Returns a[e][j] piece counts, or None."""
    m = len(caps)
    order = sorted(range(len(counts)), key=lambda e: -counts[e])

    def expert_opts(n):
        """all piece-count vectors (per class) covering n, small waste first"""
        opts = []

        def rec(j, left, used):
            if left <= 0:
                opts.append((sum(caps[k] * used[k] for k in range(m)), tuple(used)))
                return
            if j == m:
                return
            maxk = min(8, -(-left // caps[j]))
            for k in range(maxk + 1):
                used[j] = k
                rec(j + 1, left - k * caps[j], used)
            used[j] = 0

        rec(0, n, [0] * m)
        opts.sort(key=lambda o: (o[0], sum(o[1])))
        return [u for _, u in opts]

    opts = {e: expert_opts(counts[e]) if counts[e] > 0 else [tuple([0] * m)]
            for e in order}
    memo = {}

    def dfs(i, rem):
        if i == len(order):
            return []
        key = (i, rem)
        if key in memo:
            return memo[key]
        res = None
        for u in opts[order[i]]:
            nrem = tuple(r - k for r, k in zip(rem, u))
            if min(nrem) >= 0:
                sub = dfs(i + 1, nrem)
                if sub is not None:
                    res = [u] + sub
                    break
        memo[key] = res
        return res

    sol = dfs(0, tuple([8] * m))
    if sol is None:
        return None
    out = [None] * len(counts)
    for pos, e in enumerate(order):
        out[e] = sol[pos]
    return out


def kernel(x, gate_w, gate_b, w1, b1, w3, b3, w2, b2,
           sw1, sb1, sw3, sb3, sw2, sb2):
    x = np.asarray(x, dtype=np.float32)
    xt = x.reshape(T, D)

    # ---- gate (float64 host math; selection + combine weights) ----
    z = xt.astype(np.float64) @ np.asarray(gate_w, dtype=np.float64).T
    z -= z.max(axis=-1, keepdims=True)
    ez = np.exp(z)
    scores = ez / ez.sum(axis=-1, keepdims=True)          # [T, E]
    biased = scores + np.asarray(gate_b, dtype=np.float64)
    top2 = np.argsort(-biased, axis=-1, kind="stable")[:, :TOPK]   # [T, 2]
    gate_wt = np.take_along_axis(scores, top2, axis=-1).astype(np.float32)

    # expert token lists; index E is the shared expert over all tokens
    tok_idx = []
    tok_wt = []
    for e in range(E):
        sel = np.nonzero((top2 == e).any(axis=1))[0]
        we = np.where(top2[sel, 0] == e, gate_wt[sel, 0], gate_wt[sel, 1])
        tok_idx.append(sel)
        tok_wt.append(we.astype(np.float32))
    tok_idx.append(np.arange(T))
    tok_wt.append(np.ones(T, dtype=np.float32))
    counts = [len(s) for s in tok_idx]

    # ---- pick slot caps + cover assignment ----
    caps = assign = None
    for cand in CAPS_CANDIDATES:
        assign = _solve_cover(counts, cand)
        if assign is not None:
            caps = cand
            break
    while assign is None:
        cand = tuple(c + 512 for c in cand)
        assign = _solve_cover(counts, cand)
        caps = cand
    m = len(caps)

    # pieces[j] = list of (expert, lo, hi) chunks for slot class j
    pieces = {j: [] for j in range(m)}
    for e in range(E + 1):
        lo = 0
        for j in range(m):                     # caps sorted big->small
            for _ in range(assign[e][j]):
                hi = min(lo + caps[j], counts[e])
                pieces[j].append((e, lo, hi))
                lo = hi
        assert lo >= counts[e]
    for j in range(m):
        while len(pieces[j]) < N_CORES:
            pieces[j].append((0, 0, 0))

    # ---- build per-core input maps ----
    epacks = [
        _expert_pack(np.asarray(w1[e]), np.asarray(b1[e]),
                     np.asarray(w3[e]), np.asarray(b3[e]),
                     np.asarray(w2[e]), np.asarray(b2[e]))
        for e in range(E)
    ]
    epacks.append(_expert_pack(np.asarray(sw1), np.asarray(sb1),
                               np.asarray(sw3), np.asarray(sb3),
                               np.asarray(sw2), np.asarray(sb2)))
    xt16 = xt.astype(np.float16)
    in_maps = []
    for c in range(N_CORES):
        mdict = {}
        for j in range(m):
            e, lo, hi = pieces[j][c]
            mdict[f"s{j}xt"] = _xt_pack(xt16[tok_idx[e][lo:hi]], caps[j])
            for k, v in epacks[e].items():
                mdict[f"s{j}{k}"] = v
        in_maps.append(mdict)

    # ---- compile (cached) + run on all 8 cores ----
    if caps not in _kernel_cache:
        _kernel_cache[caps] = _build(caps)
    nc = _kernel_cache[caps]
    res = run_bass_kernel_spmd(nc, in_maps, list(range(N_CORES)))

    # ---- combine: weighted scatter-add of per-piece outputs ----
    out = np.zeros((T, D), dtype=np.float32)
    for c in range(N_CORES):
        for j in range(m):
            e, lo, hi = pieces[j][c]
            if hi <= lo:
                continue
            yc = res.results[c][f"s{j}y"].reshape(D, caps[j])
            idx = tok_idx[e][lo:hi]
            out[idx] += tok_wt[e][lo:hi][:, None] * yc.T[:hi - lo]
    return out.reshape(B, S, D)
